# revision 5
# baseline (speedup 1.0000x reference)
"""Trainium2 Bass kernel for nn_MultiHeadAttention_5162550690632 (v3).

B=2, S=2048, EMB=1024, H=16 heads x 64 dim. Sharding: 8 cores =
2 batches x 4 head-groups (4 heads each); every shard is independent
(tensor parallel on heads + data parallel on batch), no collectives.

v3 changes vs v2 (221.7us):
  * Scores use PE row-tiling (tile_position (0,0)/(64,0)): the two heads
    of a pair run as concurrent K=64 matmuls on the top/bottom halves of
    the array, sharing one moving stream.  2x effective score throughput
    and the block-diag kd staging + qt2 duplication DMAs die entirely
    (K/Q projections drain directly into score operand layout).
  * AV uses PE col-tiling ((0,0)/(0,64)): both heads' V*probs run
    concurrently into the two output halves of one PSUM bank.  Softmax
    denominators come from 1-column matmuls at col positions 0/32/64/96
    accumulated in a shared bank; banks with multiple accumulation
    groups are pre-zeroed by a cheap ones x zeros matmul so every group
    can run with start=False (has_written-safe).
  * z / denominator transposes moved from the PE to the DMA xbar
    (dma_start_transpose), freeing PSUM and PE cycles.
  * probs is a 4-slot ring (AV lags scores by 2 chunks), cutting SBUF.
"""

import numpy as np

import concourse.bass as bass
import concourse.mybir as mybir
import concourse.tile as tile
from concourse.tile import ScopedClock
from concourse.bass_utils import run_bass_kernel_spmd

# ---------------------------------------------------------------------------
# Workaround: this neuronxcc rejects >1 sync wait on several instruction
# encodings ("Too many sync wait commands", CoreV3GenImpl setupSyncWait).
# TileContext attaches multiple waits per instruction and its exit drain
# waits on every live processor.  Split every extra wait into a dedicated
# single-wait NOP on the same engine right before the instruction —
# per-engine queues are in-order, so this is semantically identical.

_MAX_WAITS = 1


def _legalize_multi_waits(tc):
    nc = tc.nc
    for fn in nc.m.functions:
        for bb in fn.blocks:
            snapshot = list(bb.instructions)
            if not any(
                inst.sync_info is not None
                and len(inst.sync_info.on_wait) > _MAX_WAITS
                for inst in snapshot
            ):
                continue
            created = []
            new_list = []
            for inst in snapshot:
                si = inst.sync_info
                if si is not None and len(si.on_wait) > _MAX_WAITS:
                    waits = list(si.on_wait)
                    for w in waits[_MAX_WAITS:]:
                        nop = nc.engines[inst.engine].nop(
                            nofuse=True, hint="wait_split"
                        )
                        nop.ins.sync_info = mybir.SyncInfo(
                            on_wait=[w], on_update=[]
                        )
                        created.append(nop.ins.name)
                        new_list.append(nop.ins)
                    inst.sync_info = mybir.SyncInfo(
                        on_wait=waits[:_MAX_WAITS], on_update=list(si.on_update)
                    )
                new_list.append(inst)
            cur = nc.cur_bb.bb if hasattr(nc.cur_bb, "bb") else nc.cur_bb
            if cur is not None and cur.name != bb.name:
                cur.instructions = [
                    i for i in cur.instructions if i.name not in created
                ]
            bb.instructions = new_list


def _patched_drain_and_barrier(self, tick_clock, wait_clock):
    nc = self.nc
    probe = nc.sync.nop(nofuse=True, hint="drain_probe")
    wait_clock.add_sem_waits(probe.ins, ScopedClock({None: tick_clock.global_clock}))
    waits = list(probe.ins.sync_info.on_wait)
    probe.ins.sync_info = mybir.SyncInfo(on_wait=[], on_update=[])
    name2sem = {s.name: s for s in self.sems.allocated().values()}
    for w in waits:
        nc.sync.wait_ge(name2sem[w.ant_name], w.wait_value)
    _legalize_multi_waits(self)
    nc.sync.drain()
    nc.all_engine_barrier()
    popped = nc._tile_sem_poison_stack.pop()
    assert popped is self._sem_poison
    nc.clear_and_free_semaphores(list(self.sems.allocated().values()))
    nc.all_engine_barrier()


tile.TileContext._drain_and_barrier = _patched_drain_and_barrier

# ---------------------------------------------------------------------------

F32 = mybir.dt.float32
BF16 = mybir.dt.bfloat16
AF = mybir.ActivationFunctionType
ALU = mybir.AluOpType

B, S, EMB = 2, 2048, 1024
H, DH = 16, 64
NCORES = 8
HG = 4                      # head-groups
NH = H // HG                # heads per core = 4
NP = NH // 2                # head pairs per core = 2
CH = NH * DH                # channels per core = 256
EC = EMB // 128             # EMB chunks = 8
SQT = 512                   # q-tile width
NSQ = S // SQT              # 4
NSK = S // 128              # 16 sk chunks


def _build_nc():
    nc = bass.Bass()

    xqT = nc.declare_dram_parameter("xqT", [128, NSQ, EC, SQT], BF16, isOutput=False)
    xkT = nc.declare_dram_parameter("xkT", [128, EC, S], BF16, isOutput=False)
    wqT = nc.declare_dram_parameter("wqT", [128, EC, CH], BF16, isOutput=False)
    wkT = nc.declare_dram_parameter("wkT", [128, EC, CH], BF16, isOutput=False)
    wvT = nc.declare_dram_parameter("wvT", [128, EC, CH], BF16, isOutput=False)
    bqc = nc.declare_dram_parameter("bqc", [128, 2], F32, isOutput=False)
    bkc = nc.declare_dram_parameter("bkc", [128, 2], F32, isOutput=False)
    bv = nc.declare_dram_parameter("bv", [1, CH], BF16, isOutput=False)
    maskT = nc.declare_dram_parameter("maskT", [128, NSQ, NSK, SQT], BF16, isOutput=False)
    ones_row = nc.declare_dram_parameter("ones_row", [1, 128], BF16, isOutput=False)
    out = nc.declare_dram_parameter("out", [NSQ, 4, 128, CH], F32, isOutput=True)

    with tile.TileContext(nc) as tc:
        with (
            tc.tile_pool(name="persist", bufs=1) as persist,
            tc.tile_pool(name="xqp", bufs=2) as xqp,
            tc.tile_pool(name="qp", bufs=2) as qp,
            tc.tile_pool(name="maskp", bufs=4) as maskp,
            tc.tile_pool(name="probsp", bufs=4) as probsp,
            tc.tile_pool(name="zaugp", bufs=2) as zaugp,
            tc.tile_pool(name="zTp", bufs=3) as zTp,
            tc.tile_pool(name="dCp", bufs=2) as dCp,
            tc.tile_pool(name="dTp", bufs=2) as dTp,
            tc.tile_pool(name="recipp", bufs=2) as recipp,
            tc.tile_pool(name="zsbp", bufs=2) as zsbp,
        ):
            wq_sb = persist.tile([128, EC, CH], BF16, tag="wq")
            wk_sb = persist.tile([128, EC, CH], BF16, tag="wk")
            wv_sb = persist.tile([128, EC, CH], BF16, tag="wv")
            xk_all = persist.tile([128, EC, S], BF16, tag="xk_all")
            ktile = [
                persist.tile(
                    [128, NSK, 128], BF16, tag=f"ktile{p}", name=f"ktile{p}"
                )
                for p in range(NP)
            ]
            v_sb = persist.tile([128, NSK, CH], BF16, tag="v_sb")
            ones_r = persist.tile([1, 128], BF16, tag="ones_r")
            onesP = persist.tile([128, 1], BF16, tag="onesP")
            zrow = persist.tile([1, SQT], BF16, tag="zrow")
            bq_sb = persist.tile([128, 2], F32, tag="bq")
            bk_sb = persist.tile([128, 2], F32, tag="bk")
            bv_sb = persist.tile([1, CH], BF16, tag="bv")
            warm = persist.tile([1, 8], BF16, tag="warm")

            # constants + warm the Exp table off the critical path
            nc.vector.memset(onesP[:], 1.0)
            nc.vector.memset(zrow[:], 0.0)
            nc.vector.memset(warm[:], 0.5)
            nc.scalar.activation(warm[:], warm[:], AF.Exp, scale=0.125)
            nc.gpsimd.dma_start(ones_r[:], ones_row[:])
            nc.gpsimd.dma_start(bq_sb[:], bqc[:])
            nc.gpsimd.dma_start(bk_sb[:], bkc[:])
            nc.gpsimd.dma_start(bv_sb[:], bv[:])

            xq_t = {}

            def xq_dma(sq, eng):
                xq_t[sq] = xqp.tile([128, EC, SQT], BF16, tag="xq", name=f"xq_{sq}")
                eng.dma_start(xq_t[sq][:], xqT[:, sq, :, :])

            mask_t = {}

            def mask_dma(sq, half, eng):
                key = (sq, half)
                mask_t[key] = maskp.tile(
                    [128, NSK // 2, SQT], BF16, tag="mask", name=f"mask_{sq}_{half}"
                )
                lo = half * (NSK // 2)
                eng.dma_start(mask_t[key][:], maskT[:, sq, lo : lo + NSK // 2, :])

            q_sb = {}

            # ---------------- K/Q projections, e-major ----------------
            with tc.tile_pool(name="ps_k", bufs=1, space="PSUM") as ps_k:
                kps = [
                    [
                        ps_k.tile(
                            [128, SQT], F32, tag=f"kps_{sqb}_{c}",
                            name=f"kps_{sqb}_{c}",
                        )
                        for c in range(2)
                    ]
                    for sqb in range(NSQ)
                ]
                # sync queue, priority order (cumulative per-queue deps:
                # late bulk DMAs are emitted later, inside the loops)
                nc.sync.dma_start(wk_sb[:], wkT[:])
                for e in range(EC):
                    nc.sync.dma_start(xk_all[:, e, :], xkT[:, e, :])
                nc.sync.dma_start(wq_sb[:], wqT[:])
                xq_dma(0, nc.sync)
                xq_dma(1, nc.sync)
                nc.sync.dma_start(wv_sb[:], wvT[:])
                mask_dma(0, 0, nc.sync)
                mask_dma(0, 1, nc.sync)
                mask_dma(1, 0, nc.sync)
                mask_dma(1, 1, nc.sync)

                for e in range(EC):
                    for sqb in range(NSQ):
                        ssl = slice(sqb * SQT, (sqb + 1) * SQT)
                        for c in range(2):
                            nc.tensor.matmul(
                                kps[sqb][c][:],
                                wk_sb[:, e, c * 128 : (c + 1) * 128],
                                xk_all[:, e, ssl],
                                start=(e == 0),
                                stop=(e == EC - 1),
                            )

                # drains: pair p K tile gets bias-fused cast; split DVE/ACT
                for sqb in range(NSQ):
                    for c in range(2):
                        dst = ktile[c][:, 4 * sqb : 4 * sqb + 4, :].rearrange(
                            "p a b -> p (a b)"
                        )
                        if c == 0:
                            nc.vector.tensor_scalar_add(
                                dst, kps[sqb][c][:], bk_sb[:, c : c + 1]
                            )
                        else:
                            nc.scalar.activation(
                                dst, kps[sqb][c][:], AF.Identity,
                                bias=bk_sb[:, c : c + 1],
                            )

                # Q projections for sq0/sq1 recycle the K psum tags
                def emit_q_kphase(sq):
                    q_sb[sq] = qp.tile(
                        [128, NP, SQT], BF16, tag="q", name=f"q_{sq}"
                    )
                    for p in range(NP):
                        ps = ps_k.tile(
                            [128, SQT], F32, tag=f"kps_{sq}_{p}",
                            name=f"qps_{sq}_{p}",
                        )
                        for e in range(EC):
                            nc.tensor.matmul(
                                ps[:],
                                wq_sb[:, e, p * 128 : (p + 1) * 128],
                                xq_t[sq][:, e, :],
                                start=(e == 0),
                                stop=(e == EC - 1),
                            )
                        if p == 0:
                            nc.vector.tensor_scalar_add(
                                q_sb[sq][:, p, :], ps[:], bq_sb[:, p : p + 1]
                            )
                        else:
                            nc.scalar.activation(
                                q_sb[sq][:, p, :], ps[:], AF.Identity,
                                bias=bq_sb[:, p : p + 1],
                            )

                emit_q_kphase(0)
                emit_q_kphase(1)

            # ---------------- attention ----------------
            with (
                tc.tile_pool(name="ps_s", bufs=2, space="PSUM") as ps_s,
                tc.tile_pool(name="ps_z", bufs=2, space="PSUM") as ps_z,
                tc.tile_pool(name="ps_d", bufs=1, space="PSUM") as ps_d,
                tc.tile_pool(name="ps_x", bufs=1, space="PSUM") as ps_x,
            ):
                def emit_v(t):
                    jsl = slice(t * 128, (t + 1) * 128)
                    psv = ps_x.tile([128, CH], F32, tag="vq", name=f"psv_{t}")
                    for e in range(EC):
                        nc.tensor.matmul(
                            psv[:],
                            xk_all[:, e, jsl],
                            wv_sb[:, e, :],
                            start=(e == 0),
                            stop=False,
                        )
                    nc.tensor.matmul(
                        psv[:], ones_r[:], bv_sb[:], start=False, stop=True
                    )
                    nc.vector.tensor_copy(v_sb[:, t, :], psv[:])

                def emit_q_att(sq):
                    q_sb[sq] = qp.tile(
                        [128, NP, SQT], BF16, tag="q", name=f"q_{sq}"
                    )
                    for p in range(NP):
                        ps = ps_x.tile(
                            [128, SQT], F32, tag="vq", name=f"qps_{sq}_{p}"
                        )
                        for e in range(EC):
                            nc.tensor.matmul(
                                ps[:],
                                wq_sb[:, e, p * 128 : (p + 1) * 128],
                                xq_t[sq][:, e, :],
                                start=(e == 0),
                                stop=(e == EC - 1),
                            )
                        nc.vector.tensor_scalar_add(
                            q_sb[sq][:, p, :], ps[:], bq_sb[:, p : p + 1]
                        )

                def emit_av_dn(sq, p, t, probs_t, dps, zps):
                    # col-tiled AV: both heads concurrently (out halves)
                    nc.tensor.matmul(
                        zps[0:64, :],
                        v_sb[:, t, 2 * p * DH : (2 * p + 1) * DH],
                        probs_t[:, 0, :],
                        start=False, stop=(t == NSK - 1),
                        tile_position=(0, 0), skip_group_check=True,
                    )
                    nc.tensor.matmul(
                        zps[64:128, :],
                        v_sb[:, t, (2 * p + 1) * DH : (2 * p + 2) * DH],
                        probs_t[:, 1, :],
                        start=False, stop=(t == NSK - 1),
                        tile_position=(0, 64), skip_group_check=True,
                    )
                    # denominators: 1-col matmuls at positions 64p/64p+32
                    for hip in range(2):
                        r = 64 * p + 32 * hip
                        nc.tensor.matmul(
                            dps[r : r + 1, :],
                            onesP[:],
                            probs_t[:, hip, :],
                            start=False, stop=(t == NSK - 1),
                            tile_position=(0, r), skip_group_check=True,
                        )

                for sq in range(NSQ):
                    z_sb = zsbp.tile([128, 4, CH], F32, tag="z", name=f"z_{sq}")
                    dps = ps_d.tile([128, SQT], F32, tag="d", name=f"dps_{sq}")
                    zT = {}
                    for p in range(NP):
                        ktp = ktile[p]
                        zps = ps_z.tile(
                            [128, SQT], F32, tag="zp", name=f"zps_{sq}_{p}"
                        )
                        # pre-zero banks so all accum groups can start=False
                        nc.tensor.matmul(
                            zps[:], ones_r[:], zrow[:],
                            start=True, stop=False, skip_group_check=True,
                        )
                        if p == 0:
                            nc.tensor.matmul(
                                dps[:], ones_r[:], zrow[:],
                                start=True, stop=False, skip_group_check=True,
                            )
                        pend = []
                        for t in range(NSK):
                            slot = ps_s.tile(
                                [128, 2, SQT], F32, tag="sc", name=f"sc_{t%2}"
                            )
                            nc.tensor.matmul(
                                slot[:, 0, :],
                                ktp[0:64, t, :],
                                q_sb[sq][0:64, p, :],
                                start=True, stop=True, tile_position=(0, 0),
                            )
                            nc.tensor.matmul(
                                slot[:, 1, :],
                                ktp[64:128, t, :],
                                q_sb[sq][64:128, p, :],
                                start=True, stop=True, tile_position=(64, 0),
                            )
                            # late bulk DMAs + V/Q chains interleave here
                            if sq == 0 and p == 0:
                                if t == 0:
                                    xq_dma(2, nc.gpsimd)
                                    mask_dma(2, 0, nc.gpsimd)
                                if t == 1:
                                    mask_dma(2, 1, nc.gpsimd)
                                emit_v(t)
                            if sq == 0 and p == 1 and t == 0:
                                xq_dma(3, nc.gpsimd)
                            if sq == 1 and p == 0 and t == 0:
                                mask_dma(3, 0, nc.gpsimd)
                                mask_dma(3, 1, nc.gpsimd)
                            if p == 1 and t == 2 and sq + 1 < NSQ and sq >= 1:
                                emit_q_att(sq + 1)
                            probs_t = probsp.tile(
                                [128, 2, SQT], BF16, tag="probs",
                                name=f"pr_{sq}_{p}_{t}",
                            )
                            nc.scalar.activation(
                                probs_t[:].rearrange("p a b -> p (a b)"),
                                slot[:].rearrange("p a b -> p (a b)"),
                                AF.Exp, scale=0.125,
                            )
                            m = mask_t[(sq, t // 8)][:, t % 8, :]
                            nc.vector.tensor_tensor(
                                probs_t[:],
                                probs_t[:],
                                m.unsqueeze(1).broadcast_to((128, 2, SQT)),
                                ALU.mult,
                            )
                            pend.append((t, probs_t))
                            if len(pend) > 2:
                                pt, pp = pend.pop(0)
                                emit_av_dn(sq, p, pt, pp, dps, zps)
                        for pt, pp in pend:
                            emit_av_dn(sq, p, pt, pp, dps, zps)
                        # drain z for this pair, transpose via DMA xbar
                        zaug = zaugp.tile(
                            [128, SQT], BF16, tag="zaug", name=f"za_{sq}_{p}"
                        )
                        nc.vector.tensor_copy(zaug[:], zps[:])
                        zT[p] = zTp.tile(
                            [128, 4, 128], BF16, tag="zT", name=f"zT_{sq}_{p}"
                        )
                        zt_eng = nc.scalar if (sq == NSQ - 1 and p == 1) else nc.sync
                        for j in range(4):
                            zt_eng.dma_start_transpose(
                                zT[p][:, j, :], zaug[:, j * 128 : (j + 1) * 128]
                            )
                    # sq tail: denominators -> reciprocals -> normalize -> out
                    dC = dCp.tile([128, SQT], BF16, tag="dC", name=f"dC_{sq}")
                    nc.vector.tensor_copy(dC[:], dps[:])
                    dT = dTp.tile([128, 4, 128], BF16, tag="dT", name=f"dT_{sq}")
                    dt_eng = nc.scalar if sq == NSQ - 1 else nc.sync
                    for j in range(4):
                        dt_eng.dma_start_transpose(
                            dT[:, j, :], dC[:, j * 128 : (j + 1) * 128]
                        )
                    for j in range(4):
                        recip = recipp.tile(
                            [128, 4], F32, tag="recip", name=f"rc_{sq}_{j}"
                        )
                        nc.vector.reciprocal(
                            recip[:],
                            dT[:, j, :].rearrange("p (a b) -> p a b", b=32)[:, :, 0],
                        )
                        for p in range(NP):
                            for hip in range(2):
                                h = 2 * p + hip
                                nc.vector.tensor_scalar_mul(
                                    z_sb[:, j, h * DH : (h + 1) * DH],
                                    zT[p][:, j, hip * DH : (hip + 1) * DH],
                                    recip[:, h : h + 1],
                                )
                    nc.sync.dma_start(
                        out[sq].rearrange("j p c -> p j c"), z_sb[:]
                    )

    return nc


_NC_CACHE = {}


def _get_nc():
    if "nc" not in _NC_CACHE:
        _NC_CACHE["nc"] = _build_nc()
    return _NC_CACHE["nc"]


def _prep_in_maps(x_q, x_k_v, attn_mask, w_q, b_q, w_k, b_k, w_v, b_v):
    import ml_dtypes

    bf16 = ml_dtypes.bfloat16
    x_q = np.asarray(x_q, dtype=np.float32)
    x_k_v = np.asarray(x_k_v, dtype=np.float32)
    attn_mask = np.asarray(attn_mask)
    w_q = np.asarray(w_q, dtype=np.float32)
    w_k = np.asarray(w_k, dtype=np.float32)
    w_v = np.asarray(w_v, dtype=np.float32)
    b_q = np.asarray(b_q, dtype=np.float32)
    b_k = np.asarray(b_k, dtype=np.float32)
    b_v = np.asarray(b_v, dtype=np.float32)

    # p-major layouts: every device DMA reads a contiguous per-partition
    # block (small strided segments run at ~35GB/s vs ~330GB/s contiguous)
    xqT = [
        np.ascontiguousarray(
            x_q[b].T.reshape(EC, 128, NSQ, SQT).transpose(1, 2, 0, 3)
        ).astype(bf16)
        for b in range(B)
    ]
    xkT = [
        np.ascontiguousarray(x_k_v[b].T.reshape(EC, 128, S).transpose(1, 0, 2))
        .astype(bf16)
        for b in range(B)
    ]
    maskT = [
        np.ascontiguousarray(
            (~attn_mask[b]).T.reshape(NSK, 128, NSQ, SQT).transpose(1, 2, 0, 3)
        ).astype(bf16)
        for b in range(B)
    ]
    def _wprep(w, g):
        wt = w[g * CH : (g + 1) * CH].T.reshape(EC, 128, CH).transpose(1, 0, 2)
        return np.ascontiguousarray(wt).astype(bf16)
    wqT = [_wprep(w_q, g) for g in range(HG)]
    wkT = [_wprep(w_k, g) for g in range(HG)]
    wvT = [_wprep(w_v, g) for g in range(HG)]
    # bias columns [128, 2]: bqc[p, c] = b_q[g*CH + c*128 + p]
    bqc = [
        np.ascontiguousarray(b_q[g * CH : (g + 1) * CH].reshape(2, 128).T)
        for g in range(HG)
    ]
    bkc = [
        np.ascontiguousarray(b_k[g * CH : (g + 1) * CH].reshape(2, 128).T)
        for g in range(HG)
    ]
    bvs = [
        b_v[g * CH : (g + 1) * CH].reshape(1, CH).astype(bf16) for g in range(HG)
    ]
    ones_row = np.ones((1, 128), dtype=bf16)

    in_maps = []
    for core in range(NCORES):
        b, g = divmod(core, HG)
        in_maps.append(
            {
                "xqT": xqT[b],
                "xkT": xkT[b],
                "maskT": maskT[b],
                "wqT": wqT[g],
                "wkT": wkT[g],
                "wvT": wvT[g],
                "bqc": bqc[g],
                "bkc": bkc[g],
                "bv": bvs[g],
                "ones_row": ones_row,
            }
        )
    return in_maps


def _run(inputs, **runner_kwargs):
    nc = _get_nc()
    in_maps = _prep_in_maps(**inputs)
    res = run_bass_kernel_spmd(nc, in_maps, list(range(NCORES)), **runner_kwargs)
    z = np.empty((B, S, H * DH), dtype=np.float32)
    for core in range(NCORES):
        b, g = divmod(core, HG)
        z[b, :, g * CH : (g + 1) * CH] = res.results[core]["out"].reshape(S, CH)
    return z, res


def kernel(**inputs) -> np.ndarray:
    z, _ = _run(inputs)
    return z
